# revision 43
# baseline (speedup 1.0000x reference)
"""Distributed Trainium2 kernel for causal GQA attention with RoPE.

Model: B=2, S=2048, DM=2048, H=16 q-heads, HK=4 kv-heads, D=128.
Sharding over 8 NeuronCores: core c = (batch b=c//4, kv-head kh=c%4).
Each core computes its 4 q-heads / 1 kv-head of one batch end-to-end,
AllGathers attention outputs within its 4-core batch group, and applies
a column slice of Wo, producing out[b][:, kh*512:(kh+1)*512].
"""
import contextlib
import ctypes
import os
import sys
import types

for _p in ("/opt/trn_rl_repo", "/root/.axon_site/_ro/trn_rl_repo"):
    if os.path.isdir(_p) and _p not in sys.path:
        sys.path.insert(0, _p)

import numpy as np
import ml_dtypes

import concourse.bass as bass
import concourse.mybir as mybir
import concourse.tile as tile
from concourse import bacc
from concourse.bass import ts, ds
from concourse.bass_utils import run_bass_kernel_spmd
from concourse.masks import make_identity

BF16 = ml_dtypes.bfloat16
F32 = mybir.dt.float32
BF = mybir.dt.bfloat16

B, S, DM = 2, 2048, 2048
H, HK, D = 16, 4, 128
G = H // HK          # q heads per kv head (= heads per core)
THETA = 10000.0
N_CORES = 8
KT = DM // 128       # 16 K-tiles of the model dim
TOKB = S // 128      # 16 token blocks
TCH = S // 512       # 4 token chunks of 512
HD_CORE = G * D      # 512 output dims of q per core
NEG = -1.0e30
SCALE = float(D) ** -0.5

LAST_EXEC_TIME_NS = None
LAST_RESULTS = None


# ---------------------------------------------------------------- tracing
def _install_ntff_hook():
    """Make run_bass_kernel_spmd(trace=True) work in this container."""
    try:
        from antenv.axon_hooks import get_axon_ntff_profile_hook  # noqa: F401
        return True
    except ImportError:
        pass
    so_path = "/opt/axon/libaxon_pjrt.so"
    if not os.path.exists(so_path):
        return False
    lib = ctypes.CDLL(so_path)
    if not hasattr(lib, "axon_start_nrt_profile"):
        return False
    lib.axon_start_nrt_profile.argtypes = [ctypes.POINTER(ctypes.c_int64), ctypes.c_size_t]
    lib.axon_start_nrt_profile.restype = ctypes.c_int64
    lib.axon_stop_nrt_profile.argtypes = [ctypes.c_char_p]
    lib.axon_stop_nrt_profile.restype = ctypes.c_int64

    @contextlib.contextmanager
    def _hook(output_dir, device_ids):
        import jax
        jax.devices()
        if device_ids:
            ids = (ctypes.c_int64 * len(device_ids))(*device_ids)
            rc = lib.axon_start_nrt_profile(ids, len(device_ids))
        else:
            rc = lib.axon_start_nrt_profile(None, 0)
        if rc != 0:
            raise RuntimeError(f"axon_start_nrt_profile rc={rc}")
        try:
            yield
        finally:
            n = lib.axon_stop_nrt_profile(str(output_dir).encode())
            print(f"profile: {n} file(s) in {output_dir}", file=sys.stderr)

    mod = types.ModuleType("antenv.axon_hooks")
    holder = {"h": _hook}
    mod.set_axon_ntff_profile_hook = lambda h: holder.__setitem__("h", h)
    mod.get_axon_ntff_profile_hook = lambda: holder.get("h")
    sys.modules["antenv.axon_hooks"] = mod
    import antenv
    antenv.axon_hooks = mod
    import concourse.bass_utils as bu
    bu.upload_artifacts = lambda tmpdir: str(tmpdir)
    return True



# ---------------------------------------------------------------- graph
def build_nc():
    nc = bacc.Bacc("TRN2", target_bir_lowering=False, debug=False,
                   num_devices=N_CORES)

    # x pre-packed on host as [p, kt, s] so one DMA covers a 512-token chunk
    xt = nc.dram_tensor("xt", [128, KT, S], BF, kind="ExternalInput").ap()
    wq = nc.dram_tensor("wq", [128, KT, HD_CORE], BF, kind="ExternalInput").ap()
    wk = nc.dram_tensor("wk", [128, KT, D], BF, kind="ExternalInput").ap()
    wv = nc.dram_tensor("wv", [128, KT, D], BF, kind="ExternalInput").ap()
    wo = nc.dram_tensor("wo", [128, KT, HD_CORE], BF, kind="ExternalInput").ap()
    cost = nc.dram_tensor("cost", [D, S], BF, kind="ExternalInput").ap()
    sint = nc.dram_tensor("sint", [D, S], BF, kind="ExternalInput").ap()
    pswap = nc.dram_tensor("pswap", [128, 128], BF, kind="ExternalInput").ap()
    out = nc.dram_tensor("out", [S, HD_CORE], F32, kind="ExternalOutput").ap()

    groups = [[0, 1, 2, 3], [4, 5, 6, 7]]

    with tile.TileContext(nc) as tc:
        with tc.tile_pool(name="const", bufs=1) as cpool, \
             tc.tile_pool(name="wts", bufs=1) as wpool, \
             tc.tile_pool(name="acts", bufs=1) as apool, \
             tc.tile_pool(name="xin", bufs=2) as xpool, \
             tc.tile_pool(name="work", bufs=2) as work, \
             tc.tile_pool(name="etwork", bufs=7) as etwork, \
             tc.tile_pool(name="ogp", bufs=8) as ogpool, \
             tc.tile_pool(name="stats", bufs=1) as stats, \
             tc.tile_pool(name="psmm", bufs=4, space="PSUM") as ps_mm, \
             tc.tile_pool(name="pspv", bufs=2, space="PSUM") as ps_pv, \
             tc.tile_pool(name="psden", bufs=1, space="PSUM") as ps_den, \
             tc.tile_pool(name="pswo", bufs=1, space="PSUM") as ps_wo, \
             tc.tile_pool(name="dram", bufs=1, space="DRAM") as dpool:

            # ---------------- weights + x chunks (load order = use order)
            wq_sb = wpool.tile([128, KT, HD_CORE], BF, tag="wq", name="wq")
            wk_sb = wpool.tile([128, KT, D], BF, tag="wk", name="wk")
            wv_sb = wpool.tile([128, KT, D], BF, tag="wv", name="wv")
            wo_sb = wpool.tile([128, KT, HD_CORE], BF, tag="wo", name="wo")

            def load_xc(c, nsplit=1):
                # chunk 0 is split so the first projection matmul starts as
                # soon as its slice lands; later chunks are 1 DMA
                t = xpool.tile([128, KT, 512], BF, tag="xc", name="xc")
                step = KT // nsplit
                for i in range(nsplit):
                    nc.sync.dma_start(
                        out=t[:, ds(step * i, step), :],
                        in_=xt[:, ds(step * i, step), ds(512 * c, 512)])
                return t

            # startup loads spread across engine queues: DMA triggers cost
            # ~0.6us of engine time each and issue strictly in order
            for i in range(4):
                nc.gpsimd.dma_start(out=wk_sb[:, ds(4 * i, 4), :],
                                    in_=wk[:, ds(4 * i, 4), :])
            xc_state = [load_xc(0, nsplit=8)]
            tbl = {}
            for name, src in (("cos", cost), ("sin", sint)):
                t = cpool.tile([D, S], BF, tag=name)
                nc.scalar.dma_start(out=t[:], in_=src[:])
                tbl[name] = t
            for i in range(4):
                nc.scalar.dma_start(out=wq_sb[:, ds(4 * i, 4), :],
                                    in_=wq[:, ds(4 * i, 4), :])
            nc.gpsimd.dma_start(out=wv_sb[:], in_=wv[:])
            # xc1 on the scalar DMA stream: strictly after tables+wq so the
            # startup HBM bandwidth goes to what the PE needs first
            t1 = xpool.tile([128, KT, 512], BF, tag="xc", name="xc")
            nc.scalar.dma_start(out=t1[:], in_=xt[:, :, ds(512, 512)])
            xc_state.append(t1)

            # ---------------- constants
            # transposed causal mask: keep [k_row p, q_col j] iff j >= p
            cmaskT = cpool.tile([128, 128], F32, tag="cmaskT", name="cmaskT")
            nc.gpsimd.memset(cmaskT[:], 0.0)
            nc.gpsimd.affine_select(
                out=cmaskT[:], in_=cmaskT[:],
                compare_op=mybir.AluOpType.is_ge, fill=NEG,
                base=0, pattern=[[1, 128]], channel_multiplier=-1)
            # full ones matrix: den matmul broadcasts the sum to all rows
            ones_sb = cpool.tile([128, 128], BF, tag="ones", name="ones")
            nc.gpsimd.memset(ones_sb[:], 1.0)
            pswap_sb = cpool.tile([128, 128], BF, tag="pswap", name="pswap")
            nc.gpsimd.dma_start(out=pswap_sb[:], in_=pswap[:])
            # warm up the collective path early (gpsimd-issued input DMA so
            # it doesn't queue behind the sync-engine load stream)
            warm_in = dpool.tile([128, 4], F32, tag="warm_in", name="warm_in")
            warm_out = dpool.tile([4, 128, 4], F32, tag="warm_out",
                                  name="warm_out")
            nc.gpsimd.dma_start(out=warm_in[:], in_=cost[0:128, 0:4])
            nc.gpsimd.collective_compute(
                "AllGather", mybir.AluOpType.bypass,
                replica_groups=groups,
                ins=[warm_in.opt()], outs=[warm_out.opt()])

            # ---------------- persistent activations
            qt_sb = [apool.tile([D, S], BF, tag=f"qt{h}", name=f"qt{h}")
                     for h in range(G)]
            kt_sb = apool.tile([D, S], BF, tag="kt", name="kt")
            vtok_sb = apool.tile([128, TOKB, D], BF, tag="vtok", name="vtok")

            # ---------------- projections + RoPE + v transpose
            # rope part A: drain psum to sbuf (vector) right after the mms
            def rope_a(raw_ps):
                raw = work.tile([128, 512], BF, tag="qraw", name="qraw")
                nc.vector.tensor_copy(out=raw[:], in_=raw_ps[:])
                return raw

            # rope part B (emitted one mm-group later): PE rotate + DVE mix
            def rope_b(raw, c, dst_slice):
                rot = ps_mm.tile([128, 512], F32, tag="mm", name="rot")
                nc.tensor.matmul(rot[:], pswap_sb[:], raw[:],
                                 start=True, stop=True)
                t1 = work.tile([128, 512], F32, tag="t1", name="t1")
                nc.vector.tensor_mul(t1[:], rot[:],
                                     tbl["sin"][:, ds(512 * c, 512)])
                t2 = work.tile([128, 512], F32, tag="t2", name="t2")
                nc.vector.tensor_mul(t2[:], raw[:],
                                     tbl["cos"][:, ds(512 * c, 512)])
                nc.vector.tensor_add(dst_slice, t1[:], t2[:])

            def emit_proj(c):
                xc = xc_state.pop(0)
                # mm groups in order: k, q0..q3, v; rope_b staggered one group
                pend = []   # (raw_sb, dst_slice)

                def flush_pend():
                    while pend:
                        raw, dstsl = pend.pop(0)
                        rope_b(raw, c, dstsl)

                def mm_group(w_ap, dst_slice):
                    ps = ps_mm.tile([128, 512], F32, tag="mm", name="mm")
                    for kt in range(KT):
                        nc.tensor.matmul(ps[:], w_ap[:, kt, :], xc[:, kt, :],
                                         start=(kt == 0), stop=(kt == KT - 1))
                    raw = rope_a(ps)
                    flush_pend()
                    pend.append((raw, dst_slice))

                mm_group(wk_sb, kt_sb[:, ds(512 * c, 512)])
                for h in range(G):
                    mm_group(wq_sb[:, :, ts(h, 128)],
                             qt_sb[h][:, ds(512 * c, 512)])
                # v last (no rope; transpose to token-major)
                ps = ps_mm.tile([128, 512], F32, tag="mm", name="mm")
                for kt in range(KT):
                    nc.tensor.matmul(ps[:], wv_sb[:, kt, :], xc[:, kt, :],
                                     start=(kt == 0), stop=(kt == KT - 1))
                vst = work.tile([128, 512], BF, tag="vst", name="vst")
                nc.vector.tensor_copy(out=vst[:], in_=ps[:])
                flush_pend()
                # token-major v via DMA transpose (frees the PE + a psum bank)
                for j in range(4):
                    nc.sync.dma_start_transpose(
                        out=vtok_sb[:, 4 * c + j, :], in_=vst[:, ts(j, 128)])
                if c + 2 < TCH:
                    xc_state.append(load_xc(c + 2))

            def proj_thunks(c):
                # proj chunk as filler thunks for the attn exp-bound bubbles;
                # all psum needs run serially through the single ps_wo bank
                xc = xc_state.pop(0)
                th = []
                box = {}

                def mk_group(w_ap, dst_slice, gi):
                    for i in range(8):
                        def mm2(i=i, w_ap=w_ap, gi=gi):
                            if i == 0:
                                box[gi] = ps_wo.tile([128, 512], F32,
                                                     tag="wo", name="pj")
                            for kt in (2 * i, 2 * i + 1):
                                nc.tensor.matmul(box[gi][:], w_ap[:, kt, :],
                                                 xc[:, kt, :],
                                                 start=(kt == 0),
                                                 stop=(kt == KT - 1))
                        th.append(mm2)

                    def ropeA(gi=gi):
                        raw = work.tile([128, 512], BF, tag="qraw",
                                        name="qraw")
                        nc.vector.tensor_copy(out=raw[:], in_=box[gi][:])
                        box[("r", gi)] = raw
                    th.append(ropeA)

                    def rot_th(gi=gi):
                        rot = ps_wo.tile([128, 512], F32, tag="wo",
                                         name="rot")
                        nc.tensor.matmul(rot[:], pswap_sb[:],
                                         box[("r", gi)][:],
                                         start=True, stop=True)
                        box[("o", gi)] = rot
                    th.append(rot_th)

                    def ropeB(gi=gi, dst_slice=dst_slice):
                        raw = box.pop(("r", gi))
                        rot = box.pop(("o", gi))
                        box.pop(gi)
                        t1 = work.tile([128, 512], F32, tag="t1", name="t1")
                        nc.vector.tensor_mul(t1[:], rot[:],
                                             tbl["sin"][:, ds(512 * c, 512)])
                        t2 = work.tile([128, 512], F32, tag="t2", name="t2")
                        nc.vector.tensor_mul(t2[:], raw[:],
                                             tbl["cos"][:, ds(512 * c, 512)])
                        nc.vector.tensor_add(dst_slice, t1[:], t2[:])
                    th.append(ropeB)

                mk_group(wk_sb, kt_sb[:, ds(512 * c, 512)], 0)
                for h in range(G):
                    mk_group(wq_sb[:, :, ts(h, 128)],
                             qt_sb[h][:, ds(512 * c, 512)], 1 + h)
                for i in range(8):
                    def mmv(i=i):
                        if i == 0:
                            box["v"] = ps_wo.tile([128, 512], F32,
                                                  tag="wo", name="pv")
                        for kt in (2 * i, 2 * i + 1):
                            nc.tensor.matmul(box["v"][:], wv_sb[:, kt, :],
                                             xc[:, kt, :],
                                             start=(kt == 0),
                                             stop=(kt == KT - 1))
                    th.append(mmv)

                def vdrain():
                    ps = box.pop("v")
                    vst = work.tile([128, 512], BF, tag="vst", name="vst")
                    nc.vector.tensor_copy(out=vst[:], in_=ps[:])
                    for j in range(4):
                        nc.sync.dma_start_transpose(
                            out=vtok_sb[:, 4 * c + j, :],
                            in_=vst[:, ts(j, 128)])
                th.append(vdrain)
                return th

            # ---------------- attention, scores computed pre-transposed
            # quarter t gathers all 4 local heads, tokens [512t, 512t+512);
            # quarter 3 is split into two head-pair collectives so Wo can
            # start on the first pair while heads 2/3 still compute.
            cin_q = [dpool.tile([D, G, 512], BF, tag=f"cinq{t}", name=f"cinq{t}")
                     for t in range(3)]
            cout_q = [dpool.tile([4, D, G, 512], BF, tag=f"coutq{t}",
                                 name=f"coutq{t}") for t in range(3)]
            cin_q3a = dpool.tile([D, 2, 512], BF, tag="cinq3a", name="cinq3a")
            cout_q3a = dpool.tile([4, D, 2, 512], BF, tag="coutq3a",
                                  name="coutq3a")
            cin_q3s = [dpool.tile([D, 512], BF, tag=f"cinq3s{g}",
                                  name=f"cinq3s{g}") for g in range(2)]
            cout_q3s = [dpool.tile([4, D, 512], BF, tag=f"coutq3s{g}",
                                   name=f"coutq3s{g}") for g in range(2)]

            def wo_quarter(t):
                # gathered (r, h) = global kv-head r, local head h
                # => HD K-tile index r*G + h; out rows [512t, 512t+512)
                # one DMA per peer r; srcs entries are (kt, tb -> lhsT slice)
                srcs = []
                if t < 3:
                    for r in range(4):
                        og = ogpool.tile([128, G, 512], BF, tag="og",
                                         name="og")
                        nc.sync.dma_start(out=og[:], in_=cout_q[t][r])
                        for h in range(G):
                            srcs.append((r * G + h,
                                         lambda tb, og=og, h=h:
                                         og[:, h, ds(128 * tb, 128)]))
                    srcs.sort(key=lambda kv: kv[0])
                return srcs

            def wo_q3_loads(piece):
                # piece 0: head pair 0/1 (cout_q3a); piece 1: h2; piece 2:
                # h3, token-halved (its gather is the critical tail)
                srcs = []
                if piece == 0:
                    for r in range(4):
                        og = ogpool.tile([128, 2, 512], BF, tag="og3a",
                                         name="og3a", bufs=4)
                        nc.sync.dma_start(out=og[:], in_=cout_q3a[r])
                        for hh in range(2):
                            srcs.append((r * G + hh,
                                         lambda tb, og=og, hh=hh:
                                         og[:, hh, ds(128 * tb, 128)]))
                elif piece == 1:
                    for r in range(4):
                        og = ogpool.tile([128, 512], BF, tag="og3s",
                                         name="og3s", bufs=8)
                        nc.sync.dma_start(out=og[:], in_=cout_q3s[0][r])
                        srcs.append((r * G + 2,
                                     lambda tb, og=og:
                                     og[:, ds(128 * tb, 128)]))
                else:
                    for r in range(4):
                        og = ogpool.tile([128, 512], BF, tag="og3s",
                                         name="og3s", bufs=8)
                        nc.sync.dma_start(out=og[:], in_=cout_q3s[1][r])
                        srcs.append((r * G + 3,
                                     lambda tb, og=og:
                                     og[:, ds(128 * tb, 128)]))
                return srcs

            def wo_store(t, tb, pw):
                ost = work.tile([128, 512], F32, tag="ost", name="ost",
                                bufs=4)
                nc.vector.tensor_copy(out=ost[:], in_=pw[:])
                # out stores issue from scalar so they never queue behind
                # og bulk transfers on the sync DMA queues (WAR chains)
                nc.scalar.dma_start(
                    out=out[ds(512 * t + 128 * tb, 128), :], in_=ost[:])

            def wo_mm(t, srcs):
                # tail variant: phase A/B (first 12 K-tiles) idx-major in
                # arrival order; phase C (the last-gathered head) tb-major
                # with interleaved stores so out-DMAs drain during the mms
                pws = [ps_mm.tile([128, 512], F32, tag="mm", name="mm")
                       for _ in range(4)]
                for idx in range(12):
                    kt, sl = srcs[idx]
                    for tb in range(4):
                        nc.tensor.matmul(pws[tb][:], sl(tb),
                                         wo_sb[:, kt, :],
                                         start=(idx == 0), stop=False)
                for tb in range(4):
                    for idx in range(12, 16):
                        kt, sl = srcs[idx]
                        nc.tensor.matmul(pws[tb][:], sl(tb),
                                         wo_sb[:, kt, :],
                                         start=False, stop=(idx == 15))
                    wo_store(t, tb, pws[tb])

            def wo_mm_thunks(t, srcs):
                # tb-major single-psum-bank thunk list: emitted a few matmuls
                # at a time between attention k-blocks so Wo fills the PE
                # bubbles left by the scalar-engine exp chain
                thunks = []
                box = {}

                def mk_mm(tb, idx):
                    def th():
                        if idx == 0:
                            box[tb] = ps_wo.tile([128, 512], F32, tag="wo",
                                                 name="wo")
                        kt, sl = srcs[idx]
                        nc.tensor.matmul(box[tb][:], sl(tb),
                                         wo_sb[:, kt, :],
                                         start=(idx == 0), stop=(idx == 15))
                    return th

                for tb in range(4):
                    for idx in range(16):
                        thunks.append(mk_mm(tb, idx))
                    thunks.append(lambda tb=tb: wo_store(t, tb, box.pop(tb)))
                return thunks

            def emit_st(h, qc, kb):
                """score block, transposed: [k 128, q<=512] -> exp -> et"""
                band = kb - 4 * qc
                et = etwork.tile([128, 512], BF, tag="et", name="et")
                sps = ps_mm.tile([128, 512], F32, tag="mm", name="mm")
                if band >= 0:
                    off = 128 * band
                    w = 512 - off
                    nc.tensor.matmul(sps[:, :w], kt_sb[:, ts(kb, 128)],
                                     qt_sb[h][:, ds(512 * qc + off, w)],
                                     start=True, stop=True)
                    nc.vector.tensor_add(sps[:, :128], sps[:, :128], cmaskT[:])
                    if off:
                        nc.gpsimd.memset(et[:, :off], 0.0)
                    nc.scalar.activation(
                        out=et[:, ds(off, w)], in_=sps[:, :w],
                        func=mybir.ActivationFunctionType.Exp, scale=SCALE)
                    return et, off
                nc.tensor.matmul(sps[:], kt_sb[:, ts(kb, 128)],
                                 qt_sb[h][:, ds(512 * qc, 512)],
                                 start=True, stop=True)
                nc.scalar.activation(
                    out=et[:], in_=sps[:],
                    func=mybir.ActivationFunctionType.Exp, scale=SCALE)
                return et, 0

            wo_pend = {}
            loads_at = {(2, 0): 0, (2, 3): 1, (3, 2): 2}
            enq_at = {(2, 0): 0, (3, 0): 1, (3, 3): 2}

            q3_srcs = []
            filler_q = []
            fill_skip = [0]

            def emit_fill(n):
                # warmup skip after each enqueue gives the og DMAs time to
                # land before the first filler matmul can gate the PE
                if fill_skip[0] > 0:
                    fill_skip[0] -= 1
                    return
                for _ in range(min(n, len(filler_q))):
                    filler_q.pop(0)()

            def emit_attn(qc):
                fill_rate = {1: 2, 2: 2, 3: 3}.get(qc, 0)
                for h in range(G):
                    if (qc, h) in loads_at:
                        t = loads_at[(qc, h)]
                        wo_pend[t] = wo_quarter(t)
                    if (qc, h) in enq_at:
                        t = enq_at[(qc, h)]
                        filler_q.extend(wo_mm_thunks(t, wo_pend.pop(t)))
                        fill_skip[0] = 8
                    if qc == 3 and h == 3:
                        # prefetch triggers for the pair + h2 gathers: they
                        # wait on their collective semaphores while h3's
                        # attention runs, so phase-A Wo matmuls can start the
                        # moment h3's attention ends
                        q3_srcs.extend(wo_q3_loads(0))
                        q3_srcs.extend(wo_q3_loads(1))
                    nkb = 4 * qc + 4
                    oT_ps = ps_pv.tile([128, 512], F32, tag="pv", name="pv")
                    den_ps = ps_den.tile([128, 512], F32, tag="den",
                                         name="den")
                    pend = [emit_st(h, qc, k) for k in range(min(3, nkb))]
                    ngrp = (nkb + 3) // 4
                    esum = None
                    for kb in range(nkb):
                        et, off = pend.pop(0)
                        if kb + 3 < nkb:
                            pend.append(emit_st(h, qc, kb + 3))
                        nc.tensor.matmul(oT_ps[:, ds(off, 512 - off)],
                                         vtok_sb[:, kb, :],
                                         et[:, ds(off, 512 - off)],
                                         start=(kb == 0), stop=(kb == nkb - 1))
                        # denominator: sum groups of 4 et tiles on DVE, then
                        # one ones-matmul per group (row-broadcast result)
                        gi, gj = divmod(kb, 4)
                        last_in_grp = (gj == 3 or kb == nkb - 1)
                        if gj == 0:
                            esum = et
                        else:
                            nsum = etwork.tile([128, 512], BF, tag="esum",
                                               name="esum", bufs=3)
                            nc.vector.tensor_add(nsum[:], esum[:], et[:])
                            esum = nsum
                        if last_in_grp:
                            nc.tensor.matmul(den_ps[:], ones_sb[:],
                                             esum[:],
                                             start=(gi == 0),
                                             stop=(gi == ngrp - 1))
                        emit_fill(fill_rate)
                    rec = stats.tile([128, 512], F32, tag="recq", name="recq")
                    nc.vector.reciprocal_approx_fast(out=rec[:],
                                                     in_=den_ps[:])
                    otst = work.tile([128, 512], BF, tag="otst", name="otst")
                    nc.vector.tensor_mul(otst[:], oT_ps[:], rec[:])
                    if qc < 3:
                        nc.sync.dma_start(out=cin_q[qc][:, h, :], in_=otst[:])
                    elif h < 2:
                        nc.sync.dma_start(out=cin_q3a[:, h, :], in_=otst[:])
                        if h == 1:
                            nc.gpsimd.collective_compute(
                                "AllGather", mybir.AluOpType.bypass,
                                replica_groups=groups,
                                ins=[cin_q3a.opt()], outs=[cout_q3a.opt()])
                    elif h == 2:
                        nc.sync.dma_start(out=cin_q3s[0][:], in_=otst[:])
                        nc.gpsimd.collective_compute(
                            "AllGather", mybir.AluOpType.bypass,
                            replica_groups=groups,
                            ins=[cin_q3s[0].opt()],
                            outs=[cout_q3s[0].opt()])
                    else:
                        nc.sync.dma_start(out=cin_q3s[1][:], in_=otst[:])
                        nc.gpsimd.collective_compute(
                            "AllGather", mybir.AluOpType.bypass,
                            replica_groups=groups,
                            ins=[cin_q3s[1].opt()],
                            outs=[cout_q3s[1].opt()])
                        q3_srcs.extend(wo_q3_loads(2))
                if qc < 3:
                    nc.gpsimd.collective_compute(
                        "AllGather", mybir.AluOpType.bypass,
                        replica_groups=groups,
                        ins=[cin_q[qc].opt()], outs=[cout_q[qc].opt()])

            emit_proj(0)
            emit_proj(1)
            emit_attn(0)
            filler_q.extend(proj_thunks(2))
            # wo weights: needed from the first Wo quarter
            for i in range(4):
                nc.sync.dma_start(out=wo_sb[:, ds(4 * i, 4), :],
                                  in_=wo[:, ds(4 * i, 4), :])
            emit_attn(1)
            emit_fill(len(filler_q))
            emit_proj(3)
            emit_attn(2)
            emit_attn(3)
            emit_fill(len(filler_q))
            wo_mm(3, q3_srcs)

    nc.finalize()
    return nc


_NC_CACHE = {}


def _get_nc():
    if "nc" not in _NC_CACHE:
        _NC_CACHE["nc"] = build_nc()
    return _NC_CACHE["nc"]


def _rope_tables():
    inv = 1.0 / (THETA ** (np.arange(0, D, 2, dtype=np.float64) / D))  # [64]
    pos = np.arange(S, dtype=np.float64)
    fr = pos[:, None] * inv[None, :]                 # [S, 64]
    emb = np.concatenate([fr, fr], axis=1)           # [S, D]
    cos = np.cos(emb).T.astype(np.float32)           # [D, S]
    sin = np.sin(emb).T.astype(np.float32)
    sgn = np.where(np.arange(D) < D // 2, -1.0, 1.0).astype(np.float32)[:, None]
    return cos.astype(BF16), (sin * sgn).astype(BF16)


def _pack_w(w):
    """[DM, N] -> [128, KT, N] partition-major."""
    n = w.shape[1]
    return np.ascontiguousarray(
        w.reshape(KT, 128, n).transpose(1, 0, 2)).astype(BF16)


def kernel(x, Wq, Wk, Wv, Wo):
    global LAST_EXEC_TIME_NS, LAST_RESULTS
    nc = _get_nc()
    ct, st = _rope_tables()
    psw = np.zeros((128, 128), dtype=np.float32)
    psw[(np.arange(128) + 64) % 128, np.arange(128)] = 1.0
    psw = psw.astype(BF16)
    in_maps = []
    for c in range(N_CORES):
        b, kh = c // 4, c % 4
        xp = np.ascontiguousarray(x[b].T)            # [DM, S]
        in_maps.append({
            "xt": _pack_w(xp),
            "wq": _pack_w(Wq[:, kh * HD_CORE:(kh + 1) * HD_CORE]),
            "wk": _pack_w(Wk[:, kh * D:(kh + 1) * D]),
            "wv": _pack_w(Wv[:, kh * D:(kh + 1) * D]),
            "wo": _pack_w(Wo[:, kh * HD_CORE:(kh + 1) * HD_CORE]),
            "cost": ct, "sint": st, "pswap": psw,
        })
    trace = os.environ.get("KERNEL_TRACE", "0") == "1" and _install_ntff_hook()
    res = run_bass_kernel_spmd(nc, in_maps, core_ids=list(range(N_CORES)),
                               trace=trace)
    LAST_EXEC_TIME_NS = res.exec_time_ns
    LAST_RESULTS = res
    out = np.empty((B, S, DM), dtype=np.float32)
    for c in range(N_CORES):
        b, kh = c // 4, c % 4
        out[b, :, kh * HD_CORE:(kh + 1) * HD_CORE] = res.results[c]["out"]
    return out
